# revision 42
# baseline (speedup 1.0000x reference)
"""DOTA mix E-step (vq_codebook) on 8 TRN2 NeuronCores.

out[b,k,m] = gamma_class[b,k] * softmax_m(-0.5*(log_det+maha) + log_pi)

Math: logit[b,j] = sum_d x2[b,d]*A[d,j] + sum_d x[b,d]*B[d,j]
with A = -0.5/var + c_j (fold of the per-mode constant c_j legal since
sum_d x^2 = 1), B = mu/var, c_j = -0.5*(log_det + mu'inv mu) + log_pi.
A and B are class-mean centered over each class's valid modes: a per-class
column constant cancels in the per-class softmax, which (a) bounds logits
(max per class >= 0 after the folded -4 shift: S >= e^-4, no max pass) and
(b) shrinks A/B range enough that the whole 1024-dim contraction runs in
fp8-e4m3 DoubleRow matmuls (2x PE throughput). Scales: x2*512, A*4, x*128,
B*16, so PSUM holds 2048*logit and one exp(ps/2048) dequantizes. fp8
payloads stay in [-240, 240]: the PE decodes 0xFE/0xFF as NaN.

fp8 softmax noise (~2e-2 scale-rel worst-case) is erased by an exact host
recompute of the few (row, class) pairs with gamma > 0.04 — only those can
breach the tolerance since |out err| <= gamma. That is ~8k of 4M pairs
(0.2% of the FLOPs) fused into the scatter pass.

The device returns softmax numerators e = exp(logit-4) and denominators S
(plane-summed) in one f16 tensor; the host applies gamma/S during its
scatter. Plane-major packing: classes with count>=2 sorted by mode count
descending, dealt round-robin to 8 cores; plane m holds classes with
count > m so the segmented sums are contiguous slab adds. Count-1 classes
are resp=1 on host. SPMD pad columns store A = -240 so e = exp(-60-4) = 0.

Post-GEMM ops batch over GC=2 batch chunks and run in f16. All DRAM
tensors are laid out as one contiguous run per partition per DMA (~15ns
per descriptor otherwise). DMA issues cost ~600ns on the issuing
sequencer, so they are spread across engines (x: gpsimd, out: sync).
"""

import sys

import numpy as np

sys.path.insert(0, "/opt/trn_rl_repo")

import ml_dtypes

import concourse.bass as bass
import concourse.mybir as mybir
import concourse.tile as tile
from concourse import bacc, bass_utils

F32 = mybir.dt.float32
F16 = mybir.dt.float16
F8 = mybir.dt.float8e4
NPF8 = ml_dtypes.float8_e4m3fn

B, K, M, D = 4096, 1000, 8, 512
NCORES = 8
NB = B // 128             # 32 batch chunks
GC = 2                    # chunks per post-GEMM group
EPS_REG = 1e-3
SX2 = 512.0               # x2 fp8 scale
SA = 4.0                  # A fp8 scale
SX = 128.0                # x fp8 scale
SB = 16.0                 # B fp8 scale (SX*SB == SX2*SA == SL)
SL = SX2 * SA             # = 2048: PSUM holds SL * (logit - 4)
ESHIFT = -4.0             # folded into A: e = exp(logit - 4) fits f16
PAD_A = -240.0            # pad column: logit-4 = -240*512/2048 = -60
GFIX = 0.04               # host exactly recomputes pairs with gamma > GFIX


def build_bass(planes):
    """planes: tuple n_m (m=0..7), columns per plane per core."""
    planes = [n for n in planes if n > 0]
    nv = sum(planes)
    kc = planes[0]
    poff = np.cumsum([0] + planes).tolist()   # plane offsets
    assert nv <= 1024
    # fp8 DoubleRow: moving free = 2*cw <= 512; group col tiles by PSUM bank
    banks = []
    for b0 in range(0, nv, 512):
        w = min(512, nv - b0)
        banks.append([(b0 + c0, min(256, w - c0)) for c0 in range(0, w, 256)])

    nc = bacc.Bacc("TRN2", debug=False, target_bir_lowering=False)
    NG = NB // GC
    xt = nc.dram_tensor("xt", (NG, 128, GC * 1024), F8, kind="ExternalInput")
    wa = nc.dram_tensor("wa", (128, 8 * nv), F8, kind="ExternalInput")
    nw = nv + kc              # per-chunk slab: [nv cols of e | kc cols of S]
    out = nc.dram_tensor("out", (NG, 128, GC * nw), F16,
                         kind="ExternalOutput")
    warm = nc.dram_tensor("warm", (128, 128), F32, kind="ExternalOutput")

    xt_ap, out_ap = xt.ap(), out.ap()

    # segsum planes m>=1 split into two parallel chains to halve latency and
    # balance DVE/GpSimd: DVE sums odd planes, GpSimd even planes (m>=2),
    # then one merge add
    dve_pl = [m for m in range(1, len(planes)) if m % 2 == 1]
    gp_pl = [m for m in range(2, len(planes)) if m % 2 == 0]

    with tile.TileContext(nc) as tc:
        with (
            tc.tile_pool(name="wpool", bufs=1) as wpool,
            tc.tile_pool(name="xpool", bufs=4) as xpool,
            tc.tile_pool(name="ppool", bufs=4, space="PSUM") as ppool,
            tc.tile_pool(name="epool", bufs=4) as epool,
            tc.tile_pool(name="spool", bufs=4) as spool,
        ):
            wat = wpool.tile([128, 8 * nv], F8, tag="wa")
            wa3 = wat[:].rearrange("p (i v) -> p i v", i=8)
            nc.sync.dma_start(wat[:], wa.ap()[:, :])

            # HAM warmup: dummy matmuls while DMAs land so the real GEMM
            # starts at full clock
            wz = wpool.tile([128, 128], F16, tag="warmz")
            nc.vector.memset(wz[:], 0.0)
            wps = ppool.tile([128, 1024], F32, tag="ps")
            for i in range(24):
                nc.tensor.matmul(wps[:, 0:128], lhsT=wz[:], rhs=wz[:],
                                 start=True, stop=True)
            wsb = wpool.tile([128, 128], F32, tag="warmsb")
            nc.vector.tensor_copy(wsb[:], wps[:, 0:128])
            nc.sync.dma_start(warm.ap()[:, :], wsb[:])

            for gi in range(NB // GC):
                xg = xpool.tile([128, GC * 1024], F8, tag="xg")
                nc.sync.dma_start(xg[:], xt_ap[gi])

                e4 = epool.tile([128, GC * nw], F16, tag="e4")
                for c in range(GC):
                    xa = xg[:, c * 1024:(c + 1) * 1024].rearrange(
                        "p (i j) -> p i j", i=8)
                    ps = ppool.tile([128, 1024], F32, tag="ps")
                    # start marks the whole 2KB PSUM bank pending-zero: only
                    # the first matmul in a bank carries it, and only the
                    # last matmul touching the bank carries stop.
                    for tiles in banks:
                        for ti, (c0, cw) in enumerate(tiles):
                            for g in range(4):
                                nc.tensor.matmul(
                                    ps[:, c0:c0 + cw],
                                    lhsT=xa[:, 2 * g:2 * g + 2, :],
                                    rhs=wa3[:, 2 * g:2 * g + 2, c0:c0 + cw],
                                    start=(ti == 0 and g == 0),
                                    stop=(ti == len(tiles) - 1 and g == 3),
                                    perf_mode=mybir.MatmulPerfMode.DoubleRow,
                                )
                    nc.scalar.activation(
                        e4[:, c * nw:c * nw + nv], ps[:, 0:nv],
                        mybir.ActivationFunctionType.Exp, scale=1.0 / SL)

                # segmented softmax sums into the S slab of e4, two chains:
                #   DVE:    S = p0 + p1, then += odd planes 3, 5, 7
                #   Scalar: T = copy(p2); GpSimd: T += even planes 4, 6
                #   merge:  S[:, :n2] += T
                ev = e4[:].rearrange("p (c v) -> p c v", c=GC)
                sv = ev[:, :, nv:nv + kc]
                n1 = planes[1] if len(planes) > 1 else 0
                if n1:
                    nc.vector.tensor_tensor(
                        sv[:, :, 0:n1], ev[:, :, 0:n1],
                        ev[:, :, poff[1]:poff[1] + n1],
                        op=mybir.AluOpType.add)
                for m in dve_pl:
                    if m == 1:
                        continue
                    n = planes[m]
                    nc.vector.tensor_tensor(
                        sv[:, :, 0:n], sv[:, :, 0:n],
                        ev[:, :, poff[m]:poff[m] + n],
                        op=mybir.AluOpType.add)
                if gp_pl:
                    n2 = planes[gp_pl[0]]
                    t4 = spool.tile([128, GC * n2], F16, tag="tsum")
                    tv = t4[:].rearrange("p (c k) -> p c k", c=GC)
                    nc.gpsimd.tensor_copy(
                        t4[:], ev[:, :, poff[gp_pl[0]]:poff[gp_pl[0]] + n2])
                    for m in gp_pl[1:]:
                        n = planes[m]
                        nc.gpsimd.tensor_tensor(
                            tv[:, :, 0:n], tv[:, :, 0:n],
                            ev[:, :, poff[m]:poff[m] + n],
                            op=mybir.AluOpType.add)
                    nc.gpsimd.tensor_tensor(
                        sv[:, :, 0:n2], sv[:, :, 0:n2], tv[:, :, 0:n2],
                        op=mybir.AluOpType.add)
                nc.sync.dma_start(out_ap[gi], e4[:])

    nc.compile()
    return nc


def _layout(mask):
    """Sort count>=2 classes by count desc, deal round-robin to cores.
    Returns (planes, per_core, ones): planes = SPMD-common plane sizes
    (n_m = max over cores of #{classes with count > m}); per_core = list of
    (class id array sorted desc, per-core real plane sizes)."""
    counts = mask.sum(-1).astype(int)               # (K,)
    multi = np.where(counts >= 2)[0]
    multi = multi[np.argsort(-counts[multi], kind="stable")]
    ones = np.where(counts == 1)[0]
    per_core = []
    for c in range(NCORES):
        ids = multi[c::NCORES]
        n_m = [int((counts[ids] > m).sum()) for m in range(M)]
        per_core.append((ids, n_m))
    planes = tuple(max(pc[1][m] for pc in per_core) for m in range(M))
    return planes, per_core, ones


def prep_inputs(x, gamma_class, mu_pad, var_pad, pi_pad, mask):
    x = np.asarray(x, np.float32)
    gamma_class = np.asarray(gamma_class, np.float32)
    mask = np.asarray(mask, bool)

    var = np.clip(np.asarray(var_pad, np.float64) + EPS_REG, 1e-8, None)
    inv = 1.0 / var
    logdet = np.log(var).sum(-1)                      # (K, M)
    muinv = np.asarray(mu_pad, np.float64) * inv
    muinvmu = (np.asarray(mu_pad, np.float64) * muinv).sum(-1)
    logpi = np.where(mask, np.log(np.asarray(pi_pad, np.float64) + 1e-10), 0.0)
    cmode = -0.5 * (logdet + muinvmu) + logpi         # (K, M)

    A = -0.5 * inv + cmode[..., None]                 # (K, M, D)
    Bw = muinv
    cnt = mask.sum(-1)[:, None, None].astype(np.float64)
    Am = np.where(mask[..., None], A, 0.0).sum(1, keepdims=True) / cnt
    Bm = np.where(mask[..., None], Bw, 0.0).sum(1, keepdims=True) / cnt
    # ESHIFT folded into A's columns (sum_d x2 = 1 turns it into a logit
    # shift) so e = exp(logit - 4) fits f16 without an activation bias
    Acen = A - Am + ESHIFT                            # (K, M, D) exact
    Bcen = Bw - Bm
    Ac = np.clip(Acen * SA, -240.0, 240.0)            # stored fp8
    Bc = np.clip(Bcen * SB, -240.0, 240.0)            # stored fp8

    planes, per_core, ones = _layout(mask)
    pl = [n for n in planes if n > 0]
    nv = sum(pl)
    poff = np.cumsum([0] + pl)

    NG = NB // GC
    x2 = np.clip(x.astype(np.float64) ** 2 * SX2, 0.0, 240.0)
    xs = np.clip(x.astype(np.float64) * SX, -240.0, 240.0)
    # x-aug: features 0-511 = x2 (pairs 0-1), 512-1023 = x (pairs 2-3)
    xaug = np.concatenate([x2.astype(NPF8), xs.astype(NPF8)], axis=1)
    # [gi, p, (c, i, j)] = xaug[(gi*GC + c)*128 + j, 128i + p]
    xt = np.ascontiguousarray(
        xaug.reshape(NG, GC, 128, 8, 128)
        .transpose(0, 4, 1, 3, 2).reshape(NG, 128, GC * 1024))

    in_maps = []
    for cidx in range(NCORES):
        ids, n_m = per_core[cidx]
        wa_c = np.full((nv, 2 * D), 0.0, np.float32)
        wa_c[:, :D] = PAD_A
        for m in range(len(pl)):
            n = n_m[m]
            if n:
                wa_c[poff[m]:poff[m] + n, :D] = Ac[ids[:n], m]
                wa_c[poff[m]:poff[m] + n, D:] = Bc[ids[:n], m]
        in_maps.append({
            "xt": xt,
            "wa": np.ascontiguousarray(
                wa_c.T.reshape(8, 128, nv).transpose(1, 0, 2)
                .reshape(128, 8 * nv).astype(NPF8)),
        })
    return in_maps, planes, per_core, ones, (Acen, Bcen)


_NC_CACHE = {}


def _get_nc(planes):
    if planes not in _NC_CACHE:
        _NC_CACHE[planes] = build_bass(planes)
    return _NC_CACHE[planes]


def unpack_rows(raw, width):
    """(NG, 128, GC*width) group-packed -> (B, width) float32."""
    return np.ascontiguousarray(
        np.asarray(raw).reshape(NB // GC, 128, GC, width)
        .transpose(0, 2, 1, 3)).reshape(B, width).astype(np.float32)


def scatter_core(out, e, s, gamma_class, per_core_entry, planes):
    """out[:, k, m] = gamma[:, k] * e_plane / S for one core's classes.

    e: (B, nv) softmax numerators, s: (B, kc) denominators."""
    ids, n_m = per_core_entry
    pl = [n for n in planes if n > 0]
    poff = np.cumsum([0] + pl)
    coef = gamma_class[:, ids] / s[:, :len(ids)]      # (B, len(ids))
    for m in range(len(pl)):
        n = n_m[m]
        if n:
            out[:, ids[:n], m] = e[:, poff[m]:poff[m] + n] * coef[:, :n]


def fixup_top_pairs(out, x, gamma_class, mask, Acen, Bcen):
    """Exactly recompute out[b, k] for pairs with gamma > GFIX: fp8 GEMM
    noise can only breach the tolerance where gamma is large."""
    bs, ks = np.where(gamma_class > GFIX)
    if not len(bs):
        return
    x2 = (x.astype(np.float64) ** 2)
    xf = x.astype(np.float64)
    CH = 4096
    for i0 in range(0, len(bs), CH):
        bb = bs[i0:i0 + CH]
        kk = ks[i0:i0 + CH]
        lg = (np.einsum('nd,nmd->nm', x2[bb], Acen[kk])
              + np.einsum('nd,nmd->nm', xf[bb], Bcen[kk]))   # (n, M)
        lg = np.where(mask[kk], lg, -np.inf)
        lg -= lg.max(-1, keepdims=True)
        e = np.exp(lg)
        resp = e / e.sum(-1, keepdims=True)
        out[bb, kk] = (gamma_class[bb, kk, None] * resp).astype(np.float32)


def kernel(x, gamma_class, mu_pad, var_pad, pi_pad, mask, _trace=False):
    x = np.asarray(x, np.float32)
    mask = np.asarray(mask, bool)
    in_maps, planes, per_core, ones, AB = prep_inputs(
        x, gamma_class, mu_pad, var_pad, pi_pad, mask)
    gamma_class = np.asarray(gamma_class, np.float32)
    out = np.zeros((B, K, M), np.float32)
    if len(ones):
        out[:, ones, 0] = gamma_class[:, ones]
    if sum(planes) == 0:
        return out
    nc = _get_nc(planes)
    res = bass_utils.run_bass_kernel_spmd(
        nc, in_maps, core_ids=list(range(NCORES)), trace=_trace)
    nv = sum(planes)
    kc = max(planes)
    for cidx in range(NCORES):
        es = unpack_rows(res.results[cidx]["out"], nv + kc)
        scatter_core(out, es[:, :nv], es[:, nv:], gamma_class,
                     per_core[cidx], planes)
    fixup_top_pairs(out, x, gamma_class, mask, AB[0], AB[1])
    if len(ones):
        out[:, ones, 0] = gamma_class[:, ones]
    if _trace:
        kernel.last_results = res
    return out


# revision 45
# speedup vs baseline: 1.0906x; 1.0906x over previous
"""DOTA mix E-step (vq_codebook) on 8 TRN2 NeuronCores.

out[b,k,m] = gamma_class[b,k] * softmax_m(-0.5*(log_det+maha) + log_pi)

Math: logit[b,j] = sum_d x2[b,d]*A[d,j] + sum_d x[b,d]*B[d,j]
with A = -0.5/var + c_j (fold of the per-mode constant c_j legal since
sum_d x^2 = 1), B = mu/var, c_j = -0.5*(log_det + mu'inv mu) + log_pi.
A and B are class-mean centered over each class's valid modes: a per-class
column constant cancels in the per-class softmax, which (a) bounds logits
(max per class >= 0 after the folded -4 shift: S >= e^-4, no max pass) and
(b) shrinks A/B range enough that the whole 1024-dim contraction runs in
fp8-e4m3 DoubleRow matmuls (2x PE throughput). Scales: x2*512, A*4, x*128,
B*16, so PSUM holds 2048*logit and one exp(ps/2048) dequantizes. fp8
payloads stay in [-240, 240]: the PE decodes 0xFE/0xFF as NaN.

fp8 softmax noise (~2e-2 scale-rel worst-case) is erased by an exact host
recompute of the few (row, class) pairs with gamma > 0.04 — only those can
breach the tolerance since |out err| <= gamma. That is ~8k of 4M pairs
(0.2% of the FLOPs) fused into the scatter pass.

The device returns softmax numerators e = exp(logit-4) and denominators S
(plane-summed) in one f16 tensor; the host applies gamma/S during its
scatter. Plane-major packing: classes with count>=2 sorted by mode count
descending, dealt round-robin to 8 cores; plane m holds classes with
count > m so the segmented sums are contiguous slab adds. Count-1 classes
are resp=1 on host. SPMD pad columns store A = -240 so e = exp(-60-4) = 0.

Post-GEMM ops batch over GC=2 batch chunks and run in f16. All DRAM
tensors are laid out as one contiguous run per partition per DMA (~15ns
per descriptor otherwise). DMA issues cost ~600ns on the issuing
sequencer, so they are spread across engines (x: gpsimd, out: sync).
"""

import sys

import numpy as np

sys.path.insert(0, "/opt/trn_rl_repo")

import ml_dtypes

import concourse.bass as bass
import concourse.mybir as mybir
import concourse.tile as tile
from concourse import bacc, bass_utils

F32 = mybir.dt.float32
F16 = mybir.dt.float16
F8 = mybir.dt.float8e4
NPF8 = ml_dtypes.float8_e4m3fn

B, K, M, D = 4096, 1000, 8, 512
NCORES = 8
NB = B // 128             # 32 batch chunks
GC = 2                    # chunks per post-GEMM group
EPS_REG = 1e-3
SX2 = 512.0               # x2 fp8 scale
SA = 4.0                  # A fp8 scale
SX = 128.0                # x fp8 scale
SB = 16.0                 # B fp8 scale (SX*SB == SX2*SA == SL)
SL = SX2 * SA             # = 2048: PSUM holds SL * (logit - 4)
ESHIFT = -4.0             # folded into A: e = exp(logit - 4) fits f16
PAD_A = -240.0            # pad column: logit-4 = -240*512/2048 = -60
GFIX = 0.04               # host exactly recomputes pairs with gamma > GFIX


def build_bass(planes):
    """planes: tuple n_m (m=0..7), columns per plane per core."""
    planes = [n for n in planes if n > 0]
    nv = sum(planes)
    kc = planes[0]
    poff = np.cumsum([0] + planes).tolist()   # plane offsets
    assert nv <= 1024
    # fp8 DoubleRow: moving free = 2*cw <= 512; group col tiles by PSUM bank
    banks = []
    for b0 in range(0, nv, 512):
        w = min(512, nv - b0)
        banks.append([(b0 + c0, min(256, w - c0)) for c0 in range(0, w, 256)])

    nc = bacc.Bacc("TRN2", debug=False, target_bir_lowering=False)
    NG = NB // GC
    xt = nc.dram_tensor("xt", (NG, 128, GC * 1024), F8, kind="ExternalInput")
    wa = nc.dram_tensor("wa", (8, 128, nv), F8, kind="ExternalInput")
    nw = nv + kc              # per-chunk slab: [nv cols of e | kc cols of S]
    out = nc.dram_tensor("out", (NG, 128, GC * nw), F16,
                         kind="ExternalOutput")
    warm = nc.dram_tensor("warm", (128, 128), F32, kind="ExternalOutput")

    xt_ap, out_ap = xt.ap(), out.ap()

    # segsum planes m>=1 split into two parallel chains to halve latency and
    # balance DVE/GpSimd: DVE sums odd planes, GpSimd even planes (m>=2),
    # then one merge add
    dve_pl = [m for m in range(1, len(planes)) if m % 2 == 1]
    gp_pl = [m for m in range(2, len(planes)) if m % 2 == 0]

    with tile.TileContext(nc) as tc:
        with (
            tc.tile_pool(name="wpool", bufs=1) as wpool,
            tc.tile_pool(name="xpool", bufs=4) as xpool,
            tc.tile_pool(name="ppool", bufs=4, space="PSUM") as ppool,
            tc.tile_pool(name="epool", bufs=4) as epool,
            tc.tile_pool(name="spool", bufs=4) as spool,
        ):
            wat = wpool.tile([128, 8 * nv], F8, tag="wa")
            wa3 = wat[:].rearrange("p (i v) -> p i v", i=8)
            for i in range(8):
                nc.sync.dma_start(wa3[:, i, :], wa.ap()[i])

            # HAM warmup: dummy matmuls while DMAs land so the real GEMM
            # starts at full clock
            wz = wpool.tile([128, 128], F16, tag="warmz")
            nc.vector.memset(wz[:], 0.0)
            wps = ppool.tile([128, 1024], F32, tag="ps")
            for i in range(24):
                nc.tensor.matmul(wps[:, 0:128], lhsT=wz[:], rhs=wz[:],
                                 start=True, stop=True)
            wsb = wpool.tile([128, 128], F32, tag="warmsb")
            nc.vector.tensor_copy(wsb[:], wps[:, 0:128])
            nc.sync.dma_start(warm.ap()[:, :], wsb[:])

            for gi in range(NB // GC):
                xg = xpool.tile([128, GC * 1024], F8, tag="xg")
                nc.sync.dma_start(xg[:], xt_ap[gi])

                e4 = epool.tile([128, GC * nw], F16, tag="e4")
                for c in range(GC):
                    xa = xg[:, c * 1024:(c + 1) * 1024].rearrange(
                        "p (i j) -> p i j", i=8)
                    ps = ppool.tile([128, 1024], F32, tag="ps")
                    # start marks the whole 2KB PSUM bank pending-zero: only
                    # the first matmul in a bank carries it, and only the
                    # last matmul touching the bank carries stop.
                    for tiles in banks:
                        for ti, (c0, cw) in enumerate(tiles):
                            for g in range(4):
                                nc.tensor.matmul(
                                    ps[:, c0:c0 + cw],
                                    lhsT=xa[:, 2 * g:2 * g + 2, :],
                                    rhs=wa3[:, 2 * g:2 * g + 2, c0:c0 + cw],
                                    start=(ti == 0 and g == 0),
                                    stop=(ti == len(tiles) - 1 and g == 3),
                                    perf_mode=mybir.MatmulPerfMode.DoubleRow,
                                )
                    nc.scalar.activation(
                        e4[:, c * nw:c * nw + nv], ps[:, 0:nv],
                        mybir.ActivationFunctionType.Exp, scale=1.0 / SL)

                # segmented softmax sums into the S slab of e4, two chains:
                #   DVE:    S = p0 + p1, then += odd planes 3, 5, 7
                #   Scalar: T = copy(p2); GpSimd: T += even planes 4, 6
                #   merge:  S[:, :n2] += T
                ev = e4[:].rearrange("p (c v) -> p c v", c=GC)
                sv = ev[:, :, nv:nv + kc]
                n1 = planes[1] if len(planes) > 1 else 0
                if n1:
                    nc.vector.tensor_tensor(
                        sv[:, :, 0:n1], ev[:, :, 0:n1],
                        ev[:, :, poff[1]:poff[1] + n1],
                        op=mybir.AluOpType.add)
                for m in dve_pl:
                    if m == 1:
                        continue
                    n = planes[m]
                    nc.vector.tensor_tensor(
                        sv[:, :, 0:n], sv[:, :, 0:n],
                        ev[:, :, poff[m]:poff[m] + n],
                        op=mybir.AluOpType.add)
                if gp_pl:
                    n2 = planes[gp_pl[0]]
                    t4 = spool.tile([128, GC * n2], F16, tag="tsum")
                    tv = t4[:].rearrange("p (c k) -> p c k", c=GC)
                    nc.gpsimd.tensor_copy(
                        t4[:], ev[:, :, poff[gp_pl[0]]:poff[gp_pl[0]] + n2])
                    for m in gp_pl[1:]:
                        n = planes[m]
                        nc.gpsimd.tensor_tensor(
                            tv[:, :, 0:n], tv[:, :, 0:n],
                            ev[:, :, poff[m]:poff[m] + n],
                            op=mybir.AluOpType.add)
                    nc.gpsimd.tensor_tensor(
                        sv[:, :, 0:n2], sv[:, :, 0:n2], tv[:, :, 0:n2],
                        op=mybir.AluOpType.add)
                nc.sync.dma_start(out_ap[gi], e4[:])

    nc.compile()
    return nc


def _layout(mask):
    """Sort count>=2 classes by count desc, deal round-robin to cores.
    Returns (planes, per_core, ones): planes = SPMD-common plane sizes
    (n_m = max over cores of #{classes with count > m}); per_core = list of
    (class id array sorted desc, per-core real plane sizes)."""
    counts = mask.sum(-1).astype(int)               # (K,)
    multi = np.where(counts >= 2)[0]
    multi = multi[np.argsort(-counts[multi], kind="stable")]
    ones = np.where(counts == 1)[0]
    per_core = []
    for c in range(NCORES):
        ids = multi[c::NCORES]
        n_m = [int((counts[ids] > m).sum()) for m in range(M)]
        per_core.append((ids, n_m))
    planes = tuple(max(pc[1][m] for pc in per_core) for m in range(M))
    return planes, per_core, ones


def prep_inputs(x, gamma_class, mu_pad, var_pad, pi_pad, mask):
    x = np.asarray(x, np.float32)
    gamma_class = np.asarray(gamma_class, np.float32)
    mask = np.asarray(mask, bool)

    var = np.clip(np.asarray(var_pad, np.float64) + EPS_REG, 1e-8, None)
    inv = 1.0 / var
    logdet = np.log(var).sum(-1)                      # (K, M)
    muinv = np.asarray(mu_pad, np.float64) * inv
    muinvmu = (np.asarray(mu_pad, np.float64) * muinv).sum(-1)
    logpi = np.where(mask, np.log(np.asarray(pi_pad, np.float64) + 1e-10), 0.0)
    cmode = -0.5 * (logdet + muinvmu) + logpi         # (K, M)

    A = -0.5 * inv + cmode[..., None]                 # (K, M, D)
    Bw = muinv
    cnt = mask.sum(-1)[:, None, None].astype(np.float64)
    Am = np.where(mask[..., None], A, 0.0).sum(1, keepdims=True) / cnt
    Bm = np.where(mask[..., None], Bw, 0.0).sum(1, keepdims=True) / cnt
    # ESHIFT folded into A's columns (sum_d x2 = 1 turns it into a logit
    # shift) so e = exp(logit - 4) fits f16 without an activation bias
    Acen = A - Am + ESHIFT                            # (K, M, D) exact
    Bcen = Bw - Bm
    Ac = np.clip(Acen * SA, -240.0, 240.0)            # stored fp8
    Bc = np.clip(Bcen * SB, -240.0, 240.0)            # stored fp8

    planes, per_core, ones = _layout(mask)
    pl = [n for n in planes if n > 0]
    nv = sum(pl)
    poff = np.cumsum([0] + pl)

    NG = NB // GC
    x2 = np.clip(x.astype(np.float64) ** 2 * SX2, 0.0, 240.0)
    xs = np.clip(x.astype(np.float64) * SX, -240.0, 240.0)
    # x-aug: features 0-511 = x2 (pairs 0-1), 512-1023 = x (pairs 2-3)
    xaug = np.concatenate([x2.astype(NPF8), xs.astype(NPF8)], axis=1)
    # [gi, p, (c, i, j)] = xaug[(gi*GC + c)*128 + j, 128i + p]
    xt = np.ascontiguousarray(
        xaug.reshape(NG, GC, 128, 8, 128)
        .transpose(0, 4, 1, 3, 2).reshape(NG, 128, GC * 1024))

    in_maps = []
    for cidx in range(NCORES):
        ids, n_m = per_core[cidx]
        wa_c = np.full((nv, 2 * D), 0.0, np.float32)
        wa_c[:, :D] = PAD_A
        for m in range(len(pl)):
            n = n_m[m]
            if n:
                wa_c[poff[m]:poff[m] + n, :D] = Ac[ids[:n], m]
                wa_c[poff[m]:poff[m] + n, D:] = Bc[ids[:n], m]
        in_maps.append({
            "xt": xt,
            "wa": np.ascontiguousarray(
                wa_c.T.reshape(8, 128, nv).astype(NPF8)),
        })
    return in_maps, planes, per_core, ones, (Acen, Bcen)


_NC_CACHE = {}


def _get_nc(planes):
    if planes not in _NC_CACHE:
        _NC_CACHE[planes] = build_bass(planes)
    return _NC_CACHE[planes]


def unpack_rows(raw, width):
    """(NG, 128, GC*width) group-packed -> (B, width) float32."""
    return np.ascontiguousarray(
        np.asarray(raw).reshape(NB // GC, 128, GC, width)
        .transpose(0, 2, 1, 3)).reshape(B, width).astype(np.float32)


def scatter_core(out, e, s, gamma_class, per_core_entry, planes):
    """out[:, k, m] = gamma[:, k] * e_plane / S for one core's classes.

    e: (B, nv) softmax numerators, s: (B, kc) denominators."""
    ids, n_m = per_core_entry
    pl = [n for n in planes if n > 0]
    poff = np.cumsum([0] + pl)
    coef = gamma_class[:, ids] / s[:, :len(ids)]      # (B, len(ids))
    for m in range(len(pl)):
        n = n_m[m]
        if n:
            out[:, ids[:n], m] = e[:, poff[m]:poff[m] + n] * coef[:, :n]


def fixup_top_pairs(out, x, gamma_class, mask, Acen, Bcen):
    """Exactly recompute out[b, k] for pairs with gamma > GFIX: fp8 GEMM
    noise can only breach the tolerance where gamma is large."""
    bs, ks = np.where(gamma_class > GFIX)
    if not len(bs):
        return
    x2 = (x.astype(np.float64) ** 2)
    xf = x.astype(np.float64)
    CH = 4096
    for i0 in range(0, len(bs), CH):
        bb = bs[i0:i0 + CH]
        kk = ks[i0:i0 + CH]
        lg = (np.einsum('nd,nmd->nm', x2[bb], Acen[kk])
              + np.einsum('nd,nmd->nm', xf[bb], Bcen[kk]))   # (n, M)
        lg = np.where(mask[kk], lg, -np.inf)
        lg -= lg.max(-1, keepdims=True)
        e = np.exp(lg)
        resp = e / e.sum(-1, keepdims=True)
        out[bb, kk] = (gamma_class[bb, kk, None] * resp).astype(np.float32)


def kernel(x, gamma_class, mu_pad, var_pad, pi_pad, mask, _trace=False):
    x = np.asarray(x, np.float32)
    mask = np.asarray(mask, bool)
    in_maps, planes, per_core, ones, AB = prep_inputs(
        x, gamma_class, mu_pad, var_pad, pi_pad, mask)
    gamma_class = np.asarray(gamma_class, np.float32)
    out = np.zeros((B, K, M), np.float32)
    if len(ones):
        out[:, ones, 0] = gamma_class[:, ones]
    if sum(planes) == 0:
        return out
    nc = _get_nc(planes)
    res = bass_utils.run_bass_kernel_spmd(
        nc, in_maps, core_ids=list(range(NCORES)), trace=_trace)
    nv = sum(planes)
    kc = max(planes)
    for cidx in range(NCORES):
        es = unpack_rows(res.results[cidx]["out"], nv + kc)
        scatter_core(out, es[:, :nv], es[:, nv:], gamma_class,
                     per_core[cidx], planes)
    fixup_top_pairs(out, x, gamma_class, mask, AB[0], AB[1])
    if len(ones):
        out[:, ones, 0] = gamma_class[:, ones]
    if _trace:
        kernel.last_results = res
    return out


# revision 46
# speedup vs baseline: 1.2161x; 1.1151x over previous
"""DOTA mix E-step (vq_codebook) on 8 TRN2 NeuronCores.

out[b,k,m] = gamma_class[b,k] * softmax_m(-0.5*(log_det+maha) + log_pi)

Math: logit[b,j] = sum_d x2[b,d]*A[d,j] + sum_d x[b,d]*B[d,j]
with A = -0.5/var + c_j (fold of the per-mode constant c_j legal since
sum_d x^2 = 1), B = mu/var, c_j = -0.5*(log_det + mu'inv mu) + log_pi.
A and B are class-mean centered over each class's valid modes: a per-class
column constant cancels in the per-class softmax, which (a) bounds logits
(max per class >= 0 after the folded -4 shift: S >= e^-4, no max pass) and
(b) shrinks A/B range enough that the whole 1024-dim contraction runs in
fp8-e4m3 DoubleRow matmuls (2x PE throughput). Scales: x2*512, A*4, x*128,
B*16, so PSUM holds 2048*logit and one exp(ps/2048) dequantizes. fp8
payloads stay in [-240, 240]: the PE decodes 0xFE/0xFF as NaN.

fp8 softmax noise (~2e-2 scale-rel worst-case) is erased by an exact host
recompute of the few (row, class) pairs with gamma > 0.04 — only those can
breach the tolerance since |out err| <= gamma. That is ~8k of 4M pairs
(0.2% of the FLOPs) fused into the scatter pass.

The device returns softmax numerators e = exp(logit-4) and denominators S
(plane-summed) in one f16 tensor; the host applies gamma/S during its
scatter. Plane-major packing: classes with count>=2 sorted by mode count
descending, dealt round-robin to 8 cores; plane m holds classes with
count > m so the segmented sums are contiguous slab adds. Count-1 classes
are resp=1 on host. SPMD pad columns store A = -240 so e = exp(-60-4) = 0.

Post-GEMM ops batch over GC=2 batch chunks and run in f16. All DRAM
tensors are laid out as one contiguous run per partition per DMA (~15ns
per descriptor otherwise). DMA issues cost ~600ns on the issuing
sequencer, so they are spread across engines (x: gpsimd, out: sync).
"""

import sys

import numpy as np

sys.path.insert(0, "/opt/trn_rl_repo")

import ml_dtypes

import concourse.bass as bass
import concourse.mybir as mybir
import concourse.tile as tile
from concourse import bacc, bass_utils

F32 = mybir.dt.float32
F16 = mybir.dt.float16
F8 = mybir.dt.float8e4
NPF8 = ml_dtypes.float8_e4m3fn

B, K, M, D = 4096, 1000, 8, 512
NCORES = 8
NB = B // 128             # 32 batch chunks
GC = 2                    # chunks per post-GEMM group
EPS_REG = 1e-3
SX2 = 512.0               # x2 fp8 scale
SA = 4.0                  # A fp8 scale
SX = 128.0                # x fp8 scale
SB = 16.0                 # B fp8 scale (SX*SB == SX2*SA == SL)
SL = SX2 * SA             # = 2048: PSUM holds SL * (logit - 4)
ESHIFT = -4.0             # folded into A: e = exp(logit - 4) fits f16
PAD_A = -240.0            # pad column: logit-4 = -240*512/2048 = -60
GFIX = 0.04               # host exactly recomputes pairs with gamma > GFIX


def build_bass(planes):
    """planes: tuple n_m (m=0..7), columns per plane per core."""
    planes = [n for n in planes if n > 0]
    nv = sum(planes)
    kc = planes[0]
    poff = np.cumsum([0] + planes).tolist()   # plane offsets
    assert nv <= 1024
    # fp8 DoubleRow: moving free = 2*cw <= 512; group col tiles by PSUM bank
    banks = []
    for b0 in range(0, nv, 512):
        w = min(512, nv - b0)
        banks.append([(b0 + c0, min(256, w - c0)) for c0 in range(0, w, 256)])

    nc = bacc.Bacc("TRN2", debug=False, target_bir_lowering=False)
    NG = NB // GC
    xt = nc.dram_tensor("xt", (NG, 128, GC * 1024), F8, kind="ExternalInput")
    wa = nc.dram_tensor("wa", (8, 128, nv), F8, kind="ExternalInput")
    out = nc.dram_tensor("out", (NG, 128, GC * nv), F16,
                         kind="ExternalOutput")
    warm = nc.dram_tensor("warm", (128, 128), F32, kind="ExternalOutput")

    xt_ap, out_ap = xt.ap(), out.ap()

    with tile.TileContext(nc) as tc:
        with (
            tc.tile_pool(name="wpool", bufs=1) as wpool,
            tc.tile_pool(name="xpool", bufs=4) as xpool,
            tc.tile_pool(name="ppool", bufs=4, space="PSUM") as ppool,
            tc.tile_pool(name="epool", bufs=4) as epool,
        ):
            # x for the first two groups is prefetched ahead of the 8 wa
            # issues (~600ns each on sync) so chunk 0 never waits
            xg_pre = []
            for gi in range(2):
                xg0 = xpool.tile([128, GC * 1024], F8, tag="xg")
                nc.sync.dma_start(xg0[:], xt_ap[gi])
                xg_pre.append(xg0)
            wat = wpool.tile([128, 8 * nv], F8, tag="wa")
            wa3 = wat[:].rearrange("p (i v) -> p i v", i=8)
            for i in range(8):
                nc.sync.dma_start(wa3[:, i, :], wa.ap()[i])

            # HAM warmup: dummy matmuls while DMAs land so the real GEMM
            # starts at full clock
            wz = wpool.tile([128, 128], F16, tag="warmz")
            nc.vector.memset(wz[:], 0.0)
            wps = ppool.tile([128, 1024], F32, tag="ps")
            for i in range(24):
                nc.tensor.matmul(wps[:, 0:128], lhsT=wz[:], rhs=wz[:],
                                 start=True, stop=True)
            wsb = wpool.tile([128, 128], F32, tag="warmsb")
            nc.vector.tensor_copy(wsb[:], wps[:, 0:128])
            nc.sync.dma_start(warm.ap()[:, :], wsb[:])

            for gi in range(NB // GC):
                if gi < len(xg_pre):
                    xg = xg_pre[gi]
                else:
                    xg = xpool.tile([128, GC * 1024], F8, tag="xg")
                    nc.sync.dma_start(xg[:], xt_ap[gi])

                e4 = epool.tile([128, GC * nv], F16, tag="e4")
                for c in range(GC):
                    xa = xg[:, c * 1024:(c + 1) * 1024].rearrange(
                        "p (i j) -> p i j", i=8)
                    ps = ppool.tile([128, 1024], F32, tag="ps")
                    # start marks the whole 2KB PSUM bank pending-zero: only
                    # the first matmul in a bank carries it, and only the
                    # last matmul touching the bank carries stop.
                    for tiles in banks:
                        for ti, (c0, cw) in enumerate(tiles):
                            for g in range(4):
                                nc.tensor.matmul(
                                    ps[:, c0:c0 + cw],
                                    lhsT=xa[:, 2 * g:2 * g + 2, :],
                                    rhs=wa3[:, 2 * g:2 * g + 2, c0:c0 + cw],
                                    start=(ti == 0 and g == 0),
                                    stop=(ti == len(tiles) - 1 and g == 3),
                                    perf_mode=mybir.MatmulPerfMode.DoubleRow,
                                )
                    nc.scalar.activation(
                        e4[:, c * nv:(c + 1) * nv], ps[:, 0:nv],
                        mybir.ActivationFunctionType.Exp, scale=1.0 / SL)

                nc.sync.dma_start(out_ap[gi], e4[:])

    nc.compile()
    return nc


def _layout(mask):
    """Sort count>=2 classes by count desc, deal round-robin to cores.
    Returns (planes, per_core, ones): planes = SPMD-common plane sizes
    (n_m = max over cores of #{classes with count > m}); per_core = list of
    (class id array sorted desc, per-core real plane sizes)."""
    counts = mask.sum(-1).astype(int)               # (K,)
    multi = np.where(counts >= 2)[0]
    multi = multi[np.argsort(-counts[multi], kind="stable")]
    ones = np.where(counts == 1)[0]
    per_core = []
    for c in range(NCORES):
        ids = multi[c::NCORES]
        n_m = [int((counts[ids] > m).sum()) for m in range(M)]
        per_core.append((ids, n_m))
    planes = tuple(max(pc[1][m] for pc in per_core) for m in range(M))
    return planes, per_core, ones


def prep_inputs(x, gamma_class, mu_pad, var_pad, pi_pad, mask):
    x = np.asarray(x, np.float32)
    gamma_class = np.asarray(gamma_class, np.float32)
    mask = np.asarray(mask, bool)

    var = np.clip(np.asarray(var_pad, np.float64) + EPS_REG, 1e-8, None)
    inv = 1.0 / var
    logdet = np.log(var).sum(-1)                      # (K, M)
    muinv = np.asarray(mu_pad, np.float64) * inv
    muinvmu = (np.asarray(mu_pad, np.float64) * muinv).sum(-1)
    logpi = np.where(mask, np.log(np.asarray(pi_pad, np.float64) + 1e-10), 0.0)
    cmode = -0.5 * (logdet + muinvmu) + logpi         # (K, M)

    A = -0.5 * inv + cmode[..., None]                 # (K, M, D)
    Bw = muinv
    cnt = mask.sum(-1)[:, None, None].astype(np.float64)
    Am = np.where(mask[..., None], A, 0.0).sum(1, keepdims=True) / cnt
    Bm = np.where(mask[..., None], Bw, 0.0).sum(1, keepdims=True) / cnt
    # ESHIFT folded into A's columns (sum_d x2 = 1 turns it into a logit
    # shift) so e = exp(logit - 4) fits f16 without an activation bias
    Acen = A - Am + ESHIFT                            # (K, M, D) exact
    Bcen = Bw - Bm
    Ac = np.clip(Acen * SA, -240.0, 240.0)            # stored fp8
    Bc = np.clip(Bcen * SB, -240.0, 240.0)            # stored fp8

    planes, per_core, ones = _layout(mask)
    pl = [n for n in planes if n > 0]
    nv = sum(pl)
    poff = np.cumsum([0] + pl)

    NG = NB // GC
    x2 = np.clip(x.astype(np.float64) ** 2 * SX2, 0.0, 240.0)
    xs = np.clip(x.astype(np.float64) * SX, -240.0, 240.0)
    # x-aug: features 0-511 = x2 (pairs 0-1), 512-1023 = x (pairs 2-3)
    xaug = np.concatenate([x2.astype(NPF8), xs.astype(NPF8)], axis=1)
    # [gi, p, (c, i, j)] = xaug[(gi*GC + c)*128 + j, 128i + p]
    xt = np.ascontiguousarray(
        xaug.reshape(NG, GC, 128, 8, 128)
        .transpose(0, 4, 1, 3, 2).reshape(NG, 128, GC * 1024))

    in_maps = []
    for cidx in range(NCORES):
        ids, n_m = per_core[cidx]
        wa_c = np.full((nv, 2 * D), 0.0, np.float32)
        wa_c[:, :D] = PAD_A
        for m in range(len(pl)):
            n = n_m[m]
            if n:
                wa_c[poff[m]:poff[m] + n, :D] = Ac[ids[:n], m]
                wa_c[poff[m]:poff[m] + n, D:] = Bc[ids[:n], m]
        in_maps.append({
            "xt": xt,
            "wa": np.ascontiguousarray(
                wa_c.T.reshape(8, 128, nv).astype(NPF8)),
        })
    return in_maps, planes, per_core, ones, (Acen, Bcen)


_NC_CACHE = {}


def _get_nc(planes):
    if planes not in _NC_CACHE:
        _NC_CACHE[planes] = build_bass(planes)
    return _NC_CACHE[planes]


def unpack_rows(raw, width):
    """(NG, 128, GC*width) group-packed -> (B, width) float32."""
    return np.ascontiguousarray(
        np.asarray(raw).reshape(NB // GC, 128, GC, width)
        .transpose(0, 2, 1, 3)).reshape(B, width).astype(np.float32)


def host_segsum(e, planes):
    """Per-class softmax denominators from the plane-packed numerators."""
    pl = [n for n in planes if n > 0]
    poff = np.cumsum([0] + pl)
    s = np.zeros((e.shape[0], pl[0]), np.float32)
    for m in range(len(pl)):
        s[:, :pl[m]] += e[:, poff[m]:poff[m] + pl[m]]
    return s


def scatter_core(out, e, s, gamma_class, per_core_entry, planes):
    """out[:, k, m] = gamma[:, k] * e_plane / S for one core's classes.

    e: (B, nv) softmax numerators, s: (B, kc) denominators."""
    ids, n_m = per_core_entry
    pl = [n for n in planes if n > 0]
    poff = np.cumsum([0] + pl)
    coef = gamma_class[:, ids] / s[:, :len(ids)]      # (B, len(ids))
    for m in range(len(pl)):
        n = n_m[m]
        if n:
            out[:, ids[:n], m] = e[:, poff[m]:poff[m] + n] * coef[:, :n]


def fixup_top_pairs(out, x, gamma_class, mask, Acen, Bcen):
    """Exactly recompute out[b, k] for pairs with gamma > GFIX: fp8 GEMM
    noise can only breach the tolerance where gamma is large."""
    bs, ks = np.where(gamma_class > GFIX)
    if not len(bs):
        return
    x2 = (x.astype(np.float64) ** 2)
    xf = x.astype(np.float64)
    CH = 4096
    for i0 in range(0, len(bs), CH):
        bb = bs[i0:i0 + CH]
        kk = ks[i0:i0 + CH]
        lg = (np.einsum('nd,nmd->nm', x2[bb], Acen[kk])
              + np.einsum('nd,nmd->nm', xf[bb], Bcen[kk]))   # (n, M)
        lg = np.where(mask[kk], lg, -np.inf)
        lg -= lg.max(-1, keepdims=True)
        e = np.exp(lg)
        resp = e / e.sum(-1, keepdims=True)
        out[bb, kk] = (gamma_class[bb, kk, None] * resp).astype(np.float32)


def kernel(x, gamma_class, mu_pad, var_pad, pi_pad, mask, _trace=False):
    x = np.asarray(x, np.float32)
    mask = np.asarray(mask, bool)
    in_maps, planes, per_core, ones, AB = prep_inputs(
        x, gamma_class, mu_pad, var_pad, pi_pad, mask)
    gamma_class = np.asarray(gamma_class, np.float32)
    out = np.zeros((B, K, M), np.float32)
    if len(ones):
        out[:, ones, 0] = gamma_class[:, ones]
    if sum(planes) == 0:
        return out
    nc = _get_nc(planes)
    res = bass_utils.run_bass_kernel_spmd(
        nc, in_maps, core_ids=list(range(NCORES)), trace=_trace)
    nv = sum(planes)
    for cidx in range(NCORES):
        e = unpack_rows(res.results[cidx]["out"], nv)
        scatter_core(out, e, host_segsum(e, planes), gamma_class,
                     per_core[cidx], planes)
    fixup_top_pairs(out, x, gamma_class, mask, AB[0], AB[1])
    if len(ones):
        out[:, ones, 0] = gamma_class[:, ones]
    if _trace:
        kernel.last_results = res
    return out


# revision 48
# speedup vs baseline: 1.2167x; 1.0005x over previous
"""DOTA mix E-step (vq_codebook) on 8 TRN2 NeuronCores.

out[b,k,m] = gamma_class[b,k] * softmax_m(-0.5*(log_det+maha) + log_pi)

Math: logit[b,j] = sum_d x2[b,d]*A[d,j] + sum_d x[b,d]*B[d,j]
with A = -0.5/var + c_j (fold of the per-mode constant c_j legal since
sum_d x^2 = 1), B = mu/var, c_j = -0.5*(log_det + mu'inv mu) + log_pi.
A and B are class-mean centered over each class's valid modes: a per-class
column constant cancels in the per-class softmax, which (a) bounds logits
(max per class >= 0 after the folded -4 shift: S >= e^-4, no max pass) and
(b) shrinks A/B range enough that the whole 1024-dim contraction runs in
fp8-e4m3 DoubleRow matmuls (2x PE throughput). Scales: x2*512, A*4, x*128,
B*16, so PSUM holds 2048*logit and one exp(ps/2048) dequantizes. fp8
payloads stay in [-240, 240]: the PE decodes 0xFE/0xFF as NaN.

fp8 softmax noise (~2e-2 scale-rel worst-case) is erased by an exact host
recompute of the few (row, class) pairs with gamma > 0.04 — only those can
breach the tolerance since |out err| <= gamma. That is ~8k of 4M pairs
(0.2% of the FLOPs) fused into the scatter pass.

The device is GEMM + exp + DMA only: it returns softmax numerators
e = exp(logit-4) in f16; the host forms the per-class denominators
(contiguous plane-slab sums in f32) and applies gamma/S during its
scatter. Plane-major packing: classes with count>=2 sorted by mode count
descending, dealt round-robin to 8 cores; plane m holds classes with
count > m. Count-1 classes are resp=1 on host. SPMD pad columns store
A = -240 so e = exp(-60-4) = 0.

exp batches over GC=2 batch chunks (periodic sub-us PE gaps also keep the
DVFS clock at 2.4GHz; a fully saturated PE throttles to ~1.95). All DRAM
tensors are laid out as one contiguous run per partition per DMA (~15ns
per descriptor otherwise). DMA issues cost ~600ns on the issuing
sequencer, so the first two x tiles are prefetched ahead of the 8 weight
DMA issues to keep chunk 0 from waiting.
"""

import sys

import numpy as np

sys.path.insert(0, "/opt/trn_rl_repo")

import ml_dtypes

import concourse.bass as bass
import concourse.mybir as mybir
import concourse.tile as tile
from concourse import bacc, bass_utils

F32 = mybir.dt.float32
F16 = mybir.dt.float16
F8 = mybir.dt.float8e4
NPF8 = ml_dtypes.float8_e4m3fn

B, K, M, D = 4096, 1000, 8, 512
NCORES = 8
NB = B // 128             # 32 batch chunks
GC = 2                    # chunks per post-GEMM group
EPS_REG = 1e-3
SX2 = 512.0               # x2 fp8 scale
SA = 4.0                  # A fp8 scale
SX = 128.0                # x fp8 scale
SB = 16.0                 # B fp8 scale (SX*SB == SX2*SA == SL)
SL = SX2 * SA             # = 2048: PSUM holds SL * (logit - 4)
ESHIFT = -4.0             # folded into A: e = exp(logit - 4) fits f16
PAD_A = -240.0            # pad column: logit-4 = -240*512/2048 = -60
GFIX = 0.04               # host exactly recomputes pairs with gamma > GFIX


def build_bass(planes):
    """planes: tuple n_m (m=0..7), columns per plane per core."""
    planes = [n for n in planes if n > 0]
    nv = sum(planes)
    kc = planes[0]
    poff = np.cumsum([0] + planes).tolist()   # plane offsets
    assert nv <= 1024
    # fp8 DoubleRow: moving free = 2*cw <= 512; group col tiles by PSUM bank
    banks = []
    for b0 in range(0, nv, 512):
        w = min(512, nv - b0)
        banks.append([(b0 + c0, min(256, w - c0)) for c0 in range(0, w, 256)])

    nc = bacc.Bacc("TRN2", debug=False, target_bir_lowering=False)
    NG = NB // GC
    xt = nc.dram_tensor("xt", (NG, 128, GC * 1024), F8, kind="ExternalInput")
    wa = nc.dram_tensor("wa", (8, 128, nv), F8, kind="ExternalInput")
    out = nc.dram_tensor("out", (NG, 128, GC * nv), F16,
                         kind="ExternalOutput")
    warm = nc.dram_tensor("warm", (128, 128), F32, kind="ExternalOutput")

    xt_ap, out_ap = xt.ap(), out.ap()

    with tile.TileContext(nc) as tc:
        with (
            tc.tile_pool(name="wpool", bufs=1) as wpool,
            tc.tile_pool(name="xpool", bufs=4) as xpool,
            tc.tile_pool(name="ppool", bufs=4, space="PSUM") as ppool,
            tc.tile_pool(name="epool", bufs=4) as epool,
        ):
            # x for the first two groups is prefetched ahead of the 8 wa
            # issues (~600ns each on sync) so chunk 0 never waits
            xg_pre = []
            for gi in range(3):
                xg0 = xpool.tile([128, GC * 1024], F8, tag="xg")
                nc.sync.dma_start(xg0[:], xt_ap[gi])
                xg_pre.append(xg0)
            wat = wpool.tile([128, 8 * nv], F8, tag="wa")
            wa3 = wat[:].rearrange("p (i v) -> p i v", i=8)
            for i in range(8):
                nc.sync.dma_start(wa3[:, i, :], wa.ap()[i])

            # HAM warmup: dummy matmuls while DMAs land so the real GEMM
            # starts at full clock
            wz = wpool.tile([128, 128], F16, tag="warmz")
            nc.vector.memset(wz[:], 0.0)
            wps = ppool.tile([128, 1024], F32, tag="ps")
            for i in range(24):
                nc.tensor.matmul(wps[:, 0:128], lhsT=wz[:], rhs=wz[:],
                                 start=True, stop=True)
            wsb = wpool.tile([128, 128], F32, tag="warmsb")
            nc.vector.tensor_copy(wsb[:], wps[:, 0:128])
            nc.sync.dma_start(warm.ap()[:, :], wsb[:])

            for gi in range(NB // GC):
                if gi < len(xg_pre):
                    xg = xg_pre[gi]
                else:
                    xg = xpool.tile([128, GC * 1024], F8, tag="xg")
                    nc.sync.dma_start(xg[:], xt_ap[gi])

                e4 = epool.tile([128, GC * nv], F16, tag="e4")
                for c in range(GC):
                    xa = xg[:, c * 1024:(c + 1) * 1024].rearrange(
                        "p (i j) -> p i j", i=8)
                    ps = ppool.tile([128, 1024], F32, tag="ps")
                    # start marks the whole 2KB PSUM bank pending-zero: only
                    # the first matmul in a bank carries it, and only the
                    # last matmul touching the bank carries stop.
                    for tiles in banks:
                        for ti, (c0, cw) in enumerate(tiles):
                            for g in range(4):
                                nc.tensor.matmul(
                                    ps[:, c0:c0 + cw],
                                    lhsT=xa[:, 2 * g:2 * g + 2, :],
                                    rhs=wa3[:, 2 * g:2 * g + 2, c0:c0 + cw],
                                    start=(ti == 0 and g == 0),
                                    stop=(ti == len(tiles) - 1 and g == 3),
                                    perf_mode=mybir.MatmulPerfMode.DoubleRow,
                                )
                    nc.scalar.activation(
                        e4[:, c * nv:(c + 1) * nv], ps[:, 0:nv],
                        mybir.ActivationFunctionType.Exp, scale=1.0 / SL)

                nc.sync.dma_start(out_ap[gi], e4[:])

    nc.compile()
    return nc


def _layout(mask):
    """Sort count>=2 classes by count desc, deal round-robin to cores.
    Returns (planes, per_core, ones): planes = SPMD-common plane sizes
    (n_m = max over cores of #{classes with count > m}); per_core = list of
    (class id array sorted desc, per-core real plane sizes)."""
    counts = mask.sum(-1).astype(int)               # (K,)
    multi = np.where(counts >= 2)[0]
    multi = multi[np.argsort(-counts[multi], kind="stable")]
    ones = np.where(counts == 1)[0]
    per_core = []
    for c in range(NCORES):
        ids = multi[c::NCORES]
        n_m = [int((counts[ids] > m).sum()) for m in range(M)]
        per_core.append((ids, n_m))
    planes = tuple(max(pc[1][m] for pc in per_core) for m in range(M))
    return planes, per_core, ones


def prep_inputs(x, gamma_class, mu_pad, var_pad, pi_pad, mask):
    x = np.asarray(x, np.float32)
    gamma_class = np.asarray(gamma_class, np.float32)
    mask = np.asarray(mask, bool)

    var = np.clip(np.asarray(var_pad, np.float64) + EPS_REG, 1e-8, None)
    inv = 1.0 / var
    logdet = np.log(var).sum(-1)                      # (K, M)
    muinv = np.asarray(mu_pad, np.float64) * inv
    muinvmu = (np.asarray(mu_pad, np.float64) * muinv).sum(-1)
    logpi = np.where(mask, np.log(np.asarray(pi_pad, np.float64) + 1e-10), 0.0)
    cmode = -0.5 * (logdet + muinvmu) + logpi         # (K, M)

    A = -0.5 * inv + cmode[..., None]                 # (K, M, D)
    Bw = muinv
    cnt = mask.sum(-1)[:, None, None].astype(np.float64)
    Am = np.where(mask[..., None], A, 0.0).sum(1, keepdims=True) / cnt
    Bm = np.where(mask[..., None], Bw, 0.0).sum(1, keepdims=True) / cnt
    # ESHIFT folded into A's columns (sum_d x2 = 1 turns it into a logit
    # shift) so e = exp(logit - 4) fits f16 without an activation bias
    Acen = A - Am + ESHIFT                            # (K, M, D) exact
    Bcen = Bw - Bm
    Ac = np.clip(Acen * SA, -240.0, 240.0)            # stored fp8
    Bc = np.clip(Bcen * SB, -240.0, 240.0)            # stored fp8

    planes, per_core, ones = _layout(mask)
    pl = [n for n in planes if n > 0]
    nv = sum(pl)
    poff = np.cumsum([0] + pl)

    NG = NB // GC
    x2 = np.clip(x.astype(np.float64) ** 2 * SX2, 0.0, 240.0)
    xs = np.clip(x.astype(np.float64) * SX, -240.0, 240.0)
    # x-aug: features 0-511 = x2 (pairs 0-1), 512-1023 = x (pairs 2-3)
    xaug = np.concatenate([x2.astype(NPF8), xs.astype(NPF8)], axis=1)
    # [gi, p, (c, i, j)] = xaug[(gi*GC + c)*128 + j, 128i + p]
    xt = np.ascontiguousarray(
        xaug.reshape(NG, GC, 128, 8, 128)
        .transpose(0, 4, 1, 3, 2).reshape(NG, 128, GC * 1024))

    in_maps = []
    for cidx in range(NCORES):
        ids, n_m = per_core[cidx]
        wa_c = np.full((nv, 2 * D), 0.0, np.float32)
        wa_c[:, :D] = PAD_A
        for m in range(len(pl)):
            n = n_m[m]
            if n:
                wa_c[poff[m]:poff[m] + n, :D] = Ac[ids[:n], m]
                wa_c[poff[m]:poff[m] + n, D:] = Bc[ids[:n], m]
        in_maps.append({
            "xt": xt,
            "wa": np.ascontiguousarray(
                wa_c.T.reshape(8, 128, nv).astype(NPF8)),
        })
    return in_maps, planes, per_core, ones, (Acen, Bcen)


_NC_CACHE = {}


def _get_nc(planes):
    if planes not in _NC_CACHE:
        _NC_CACHE[planes] = build_bass(planes)
    return _NC_CACHE[planes]


def unpack_rows(raw, width):
    """(NG, 128, GC*width) group-packed -> (B, width) float32."""
    return np.ascontiguousarray(
        np.asarray(raw).reshape(NB // GC, 128, GC, width)
        .transpose(0, 2, 1, 3)).reshape(B, width).astype(np.float32)


def host_segsum(e, planes):
    """Per-class softmax denominators from the plane-packed numerators."""
    pl = [n for n in planes if n > 0]
    poff = np.cumsum([0] + pl)
    s = np.zeros((e.shape[0], pl[0]), np.float32)
    for m in range(len(pl)):
        s[:, :pl[m]] += e[:, poff[m]:poff[m] + pl[m]]
    return s


def scatter_core(out, e, s, gamma_class, per_core_entry, planes):
    """out[:, k, m] = gamma[:, k] * e_plane / S for one core's classes.

    e: (B, nv) softmax numerators, s: (B, kc) denominators."""
    ids, n_m = per_core_entry
    pl = [n for n in planes if n > 0]
    poff = np.cumsum([0] + pl)
    coef = gamma_class[:, ids] / s[:, :len(ids)]      # (B, len(ids))
    for m in range(len(pl)):
        n = n_m[m]
        if n:
            out[:, ids[:n], m] = e[:, poff[m]:poff[m] + n] * coef[:, :n]


def fixup_top_pairs(out, x, gamma_class, mask, Acen, Bcen):
    """Exactly recompute out[b, k] for pairs with gamma > GFIX: fp8 GEMM
    noise can only breach the tolerance where gamma is large."""
    bs, ks = np.where(gamma_class > GFIX)
    if not len(bs):
        return
    x2 = (x.astype(np.float64) ** 2)
    xf = x.astype(np.float64)
    CH = 4096
    for i0 in range(0, len(bs), CH):
        bb = bs[i0:i0 + CH]
        kk = ks[i0:i0 + CH]
        lg = (np.einsum('nd,nmd->nm', x2[bb], Acen[kk])
              + np.einsum('nd,nmd->nm', xf[bb], Bcen[kk]))   # (n, M)
        lg = np.where(mask[kk], lg, -np.inf)
        lg -= lg.max(-1, keepdims=True)
        e = np.exp(lg)
        resp = e / e.sum(-1, keepdims=True)
        out[bb, kk] = (gamma_class[bb, kk, None] * resp).astype(np.float32)


def kernel(x, gamma_class, mu_pad, var_pad, pi_pad, mask, _trace=False):
    x = np.asarray(x, np.float32)
    mask = np.asarray(mask, bool)
    in_maps, planes, per_core, ones, AB = prep_inputs(
        x, gamma_class, mu_pad, var_pad, pi_pad, mask)
    gamma_class = np.asarray(gamma_class, np.float32)
    out = np.zeros((B, K, M), np.float32)
    if len(ones):
        out[:, ones, 0] = gamma_class[:, ones]
    if sum(planes) == 0:
        return out
    nc = _get_nc(planes)
    res = bass_utils.run_bass_kernel_spmd(
        nc, in_maps, core_ids=list(range(NCORES)), trace=_trace)
    nv = sum(planes)
    for cidx in range(NCORES):
        e = unpack_rows(res.results[cidx]["out"], nv)
        scatter_core(out, e, host_segsum(e, planes), gamma_class,
                     per_core[cidx], planes)
    fixup_top_pairs(out, x, gamma_class, mask, AB[0], AB[1])
    if len(ones):
        out[:, ones, 0] = gamma_class[:, ones]
    if _trace:
        kernel.last_results = res
    return out
